# revision 1
# baseline (speedup 1.0000x reference)
"""Bahdanau-attention kernel for 8 TRN2 NeuronCores.

Reference computation (B=32, S=2048, H=1024):
    eo   = encoder_outputs.transpose(1,0,2)            # [B,S,H]
    z    = hidden @ W[:, :H].T + eo @ W[:, H:].T + b   # [B,S,H]  (split concat)
    s    = tanh(z)
    sc   = einsum('bsh,h->bs', s, v)
    sc   = where(mask, -1e9, sc); softmax over S       # [B,1,S]

Sharding: data-parallel over batch, 4 batches per core, no collectives.
Per core: z_eo = We @ eo_b^T as [h, s] tiles on TensorE (bf16, fp32 psum),
tanh + per-(h,b) bias fused on ScalarE, v-weighted accumulate on VectorE,
partition-reduce via ones-matmul, chunked masked softmax on-device.

Softmax skips the max-subtraction: |score| <= sum|v| ~ 16, so exp() stays
comfortably inside fp32 range, and masked lanes see exp(-1e30) == 0.
"""

import sys

if "/opt/trn_rl_repo" not in sys.path:
    sys.path.insert(0, "/opt/trn_rl_repo")

import numpy as np

B, S, H = 32, 2048, 1024
NCORES = 8
BL = B // NCORES          # batches per core = 4
P = 128                   # partitions
KT = H // P               # k-tiles over the contraction dim = 8
HT = H // P               # h-tiles over the attn output dim = 8
ST = 512                  # s-tile (psum bank width in fp32)
NS = S // ST              # s-tiles per batch = 4

_compiled_nc = None


def _build():
    import concourse.mybir as mybir
    from concourse import tile, bacc
    from concourse.tile import add_dep_helper

    f32 = mybir.dt.float32
    bf16 = mybir.dt.bfloat16
    u8 = mybir.dt.uint8
    AF = mybir.ActivationFunctionType
    ALU = mybir.AluOpType
    AX = mybir.AxisListType

    nc = bacc.Bacc("TRN2", target_bir_lowering=False, debug=False,
                   num_devices=NCORES)

    eoT = nc.dram_tensor("eoT", [BL, H, S], bf16, kind="ExternalInput")
    wT = nc.dram_tensor("wT", [2 * H, H], bf16, kind="ExternalInput")
    hTr = nc.dram_tensor("hTr", [P, KT, BL], bf16, kind="ExternalInput")
    biasr = nc.dram_tensor("biasr", [P, HT], f32, kind="ExternalInput")
    vr = nc.dram_tensor("vr", [P, HT], f32, kind="ExternalInput")
    mask = nc.dram_tensor("mask", [BL, S], u8, kind="ExternalInput")
    out = nc.dram_tensor("out", [BL, S], f32, kind="ExternalOutput")

    with tile.TileContext(nc) as tc:
        with (
            tc.tile_pool(name="const", bufs=1) as const,
            tc.tile_pool(name="eo", bufs=8) as eo_pool,
            tc.tile_pool(name="tpool", bufs=6) as t_pool,
            tc.tile_pool(name="tvpool", bufs=6) as tv_pool,
            tc.tile_pool(name="accpool", bufs=6) as acc_pool,
            tc.tile_pool(name="scpool", bufs=4) as sc_pool,
            tc.tile_pool(name="mskpool", bufs=4) as msk_pool,
            tc.tile_pool(name="psz", bufs=7, space="PSUM") as psum_z,
            tc.tile_pool(name="pss", bufs=1, space="PSUM") as psum_s,
        ):
            # --- tiny consts land first (HWDGE), gate ScalarE/pre ---
            hT_sb = const.tile([P, KT, BL], bf16)
            nc.sync.dma_start(hT_sb[:], hTr[:, :, :])
            bias_sb = const.tile([P, HT], f32)
            nc.sync.dma_start(bias_sb[:], biasr[:, :])
            v_sb = const.tile([P, HT], f32)
            nc.sync.dma_start(v_sb[:], vr[:, :])
            mask_row = const.tile([1, BL * S], u8)
            nc.sync.dma_start(mask_row[:],
                              mask.rearrange("b s -> (b s)")[None, :])

            ones_sb = const.tile([P, 1], bf16)
            nc.any.memset(ones_sb[:], 1.0)
            junk = const.tile([P, ST], bf16)
            nc.vector.tensor_copy(junk[:, 0:1], ones_sb[:])

            # weights ride the HWDGE ring (sub-us first byte, FIFO among
            # themselves); the first eo tile rides SWDGE concurrently.
            # Later eo prefetches chain behind we0 so the head window only
            # ever has a couple of streams splitting HBM bandwidth (the 16
            # SDMA engines drain all queued jobs round-robin otherwise).
            wh_sb = const.tile([P, KT, H], bf16)
            nc.sync.dma_start(
                wh_sb[:], wT[0:H, :].rearrange("(kk p) h -> p kk h", p=P))
            eo_first = eo_pool.tile([P, KT, ST], bf16, tag="eo")
            nc.gpsimd.dma_start(
                eo_first[:],
                eoT[0, :, 0:ST].rearrange("(kk p) s -> p kk s", p=P))
            we_sb = const.tile([P, KT, H], bf16)
            d_we0 = nc.sync.dma_start(
                we_sb[:, :, 0:H // 2],
                wT[H:2 * H, 0:H // 2].rearrange("(kk p) h -> p kk h", p=P))
            nc.sync.dma_start(
                we_sb[:, :, H // 2:H],
                wT[H:2 * H, H // 2:H].rearrange("(kk p) h -> p kk h", p=P))
            _dma_chain = [d_we0]

            mneg_row = const.tile([1, BL * S], f32)
            nc.vector.tensor_scalar(mneg_row[:], mask_row[:], -1e30, None,
                                    ALU.mult)

            # PE warmup: dummy matmuls ride out the HAM cold window while
            # the weight/eo DMAs stream in.
            wps = psum_z.tile([P, ST], f32, tag="psz")
            for w in range(48):
                nc.tensor.matmul(wps[:], junk[:, 0:P], junk[:],
                                 start=(w == 0), stop=(w == 47),
                                 skip_group_check=True)

            # pre[h, b] = (hidden @ Wh^T)[b, h] + bias[h], h on partitions.
            pre_sb = const.tile([P, HT * BL], f32)
            for hh in range(HT):
                ps = psum_z.tile([P, ST], f32, tag="psz")
                for kk in range(KT):
                    nc.tensor.matmul(
                        ps[:, :BL],
                        wh_sb[:, kk, hh * P:(hh + 1) * P],
                        hT_sb[:, kk, :],
                        start=(kk == 0), stop=(kk == KT - 1))
                nc.scalar.activation(pre_sb[:, hh * BL:(hh + 1) * BL],
                                     ps[:, :BL], AF.Identity,
                                     bias=bias_sb[:, hh:hh + 1])

            # second warmup burst: keeps the PE busy between `pre` and the
            # arrival of the first eo tile
            wps2 = psum_z.tile([P, ST], f32, tag="psz")
            for w in range(8):
                nc.tensor.matmul(wps2[:], junk[:, 0:P], junk[:],
                                 start=(w == 0), stop=(w == 7),
                                 skip_group_check=True)

            e_sb = const.tile([BL, S], f32)
            o_sb = const.tile([BL, S], f32)
            red_row = const.tile([1, BL * NS + 2], f32)
            psums4 = const.tile([BL, NS + 2], f32)
            nc.vector.memset(psums4[:], 0.0)

            def flush_scores(pends):
                if not pends:
                    return
                # pack up to 4 M=1 ones-matmuls into distinct 32-row column
                # groups of one psum bank -- they run concurrently on the PE
                pssc = psum_s.tile([P, ST], f32, tag="pss")
                for j, (acc_p, _, _) in enumerate(pends):
                    nc.tensor.matmul(pssc[32 * j:32 * j + 1], ones_sb[:],
                                     acc_p[:], start=True, stop=True,
                                     tile_position=(0, 32 * j))
                for j, (_, b_p, si_p) in enumerate(pends):
                    row = pssc[32 * j:32 * j + 1]
                    # mask + exp on the row, partial sum via accum
                    sc_m = sc_pool.tile([1, ST], f32, tag="sc")
                    off = b_p * S + si_p * ST
                    nc.vector.tensor_tensor(sc_m[:], row,
                                            mneg_row[:, off:off + ST],
                                            ALU.add)
                    e_row = msk_pool.tile([1, ST], f32, tag="m")
                    idx = b_p * NS + si_p
                    nc.scalar.activation(e_row[:], sc_m[:], AF.Exp,
                                         accum_out=red_row[:, idx:idx + 1])
                    nc.sync.dma_start(
                        e_sb[b_p:b_p + 1, si_p * ST:(si_p + 1) * ST],
                        e_row[:])
                    # scatter the partial sum to its batch partition now
                    nc.sync.dma_start(psums4[b_p:b_p + 1, si_p:si_p + 1],
                                      red_row[:, idx:idx + 1])

            pending = []
            n_groups = [0]
            # batch-major: each batch's scores finalize while the next
            # batch computes, so rows 0-2 normalize + store early.
            for b in range(BL):
                for si in range(NS):
                    if b == 0 and si == 0:
                        eo_sb = eo_first
                    else:
                        eo_sb = eo_pool.tile([P, KT, ST], bf16, tag="eo")
                        d_eo = nc.gpsimd.dma_start(
                            eo_sb[:],
                            eoT[b, :, si * ST:(si + 1) * ST].rearrange(
                                "(kk p) s -> p kk s", p=P))
                        if len(_dma_chain) < 4:
                            add_dep_helper(d_eo.ins, _dma_chain[-1].ins, True,
                                           "serial head dma")
                            _dma_chain.append(d_eo)
                    last = (si == NS - 1 and b == BL - 1)
                    halves = (ST,) if not last else (3 * ST // 4, ST // 4)
                    hoff = 0
                    for hf, HW_ in enumerate(halves):
                        hs = slice(hoff, hoff + HW_)
                        acc = acc_pool.tile([P, HW_], bf16, tag="acc")
                        for hh in range(HT):
                            ps = psum_z.tile([P, HW_], f32, tag="psz")
                            for kk in range(KT):
                                nc.tensor.matmul(
                                    ps[:],
                                    we_sb[:, kk, hh * P:(hh + 1) * P],
                                    eo_sb[:, kk, hs],
                                    start=(kk == 0), stop=(kk == KT - 1))
                            if hh == 3 and hf == 0 and (
                                    len(pending) == 4 or last):
                                flush_scores(pending)
                                pending = []
                                n_groups[0] += 1
                                if n_groups[0] == 3:
                                    # batches 0-2 complete: normalize and
                                    # store their rows under b3's compute
                                    r3 = const.tile([3, 1], f32)
                                    nc.vector.reduce_sum(r3[:],
                                                         psums4[0:3, :],
                                                         axis=AX.X)
                                    nc.vector.reciprocal(r3[:], r3[:])
                                    nc.vector.tensor_scalar(
                                        o_sb[0:3, :], e_sb[0:3, :], r3[:],
                                        None, ALU.mult)
                                    nc.sync.dma_start(out[0:3, :],
                                                      o_sb[0:3, :])
                            t_sb = t_pool.tile([P, HW_], bf16, tag="t")
                            nc.scalar.activation(
                                t_sb[:], ps[:], AF.Tanh,
                                bias=pre_sb[:, hh * BL + b:hh * BL + b + 1])
                            if hh == 0:
                                nc.vector.tensor_scalar(acc[:], t_sb[:],
                                                        v_sb[:, 0:1], None,
                                                        ALU.mult)
                            else:
                                tv = tv_pool.tile([P, HW_], bf16, tag="tv")
                                nc.vector.tensor_scalar(tv[:], t_sb[:],
                                                        v_sb[:, hh:hh + 1],
                                                        None, ALU.mult)
                                nc.vector.tensor_tensor(acc[:], acc[:],
                                                        tv[:], ALU.add)
                        if not last:
                            pending.append((acc, b, si))
                        else:
                            # inline flush of the half-tile, minimal chain
                            pssc = psum_s.tile([P, ST], f32, tag="pss")
                            nc.tensor.matmul(pssc[:1, :HW_], ones_sb[:],
                                             acc[:], start=True, stop=True)
                            sc_m = sc_pool.tile([1, HW_], f32, tag="sc")
                            off = b * S + si * ST + hoff
                            nc.vector.tensor_tensor(
                                sc_m[:], pssc[:1, :HW_],
                                mneg_row[:, off:off + HW_], ALU.add)
                            e_row = msk_pool.tile([1, HW_], f32, tag="m")
                            idx = BL * NS + hf
                            nc.scalar.activation(
                                e_row[:], sc_m[:], AF.Exp,
                                accum_out=red_row[:, idx:idx + 1])
                            nc.sync.dma_start(
                                psums4[b:b + 1, NS + hf:NS + hf + 1],
                                red_row[:, idx:idx + 1])
                            nc.sync.dma_start(
                                e_sb[b:b + 1, off - b * S:off - b * S + HW_],
                                e_row[:])
                        hoff += HW_

            # tail: only batch 3 is left (rows 0-2 already stored). Engine
            # ops must start at partition 0, so compute [4, S] (rows 0-2
            # recompute to identical values) but store only row 3.
            rinv4 = const.tile([BL, 1], f32)
            nc.vector.reduce_sum(rinv4[:], psums4[:], axis=AX.X)
            nc.vector.reciprocal(rinv4[:], rinv4[:])
            for ci in range(4):
                cs = slice(ci * (S // 4), (ci + 1) * (S // 4))
                nc.vector.tensor_scalar(o_sb[:, cs], e_sb[:, cs], rinv4[:],
                                        None, ALU.mult)
                nc.sync.dma_start(out[3:4, cs], o_sb[3:4, cs])

    nc.compile()
    return nc


def _get_nc():
    global _compiled_nc
    if _compiled_nc is None:
        _compiled_nc = _build()
    return _compiled_nc


def _make_in_maps(hidden, encoder_outputs, encoder_mask, W, b, v):
    import ml_dtypes

    bf16 = ml_dtypes.bfloat16
    hidden = np.asarray(hidden, dtype=np.float32)
    encoder_outputs = np.asarray(encoder_outputs, dtype=np.float32)
    W = np.asarray(W, dtype=np.float32)
    b = np.asarray(b, dtype=np.float32)
    v = np.asarray(v, dtype=np.float32)
    mask_u8 = np.asarray(encoder_mask).reshape(B, S).astype(np.uint8)

    # [S, B, H] -> [B, H, S] so the contraction dim lands on partitions;
    # bf16 so the kernel streams half the bytes (matmuls run in bf16 anyway)
    eoT = np.ascontiguousarray(encoder_outputs.transpose(1, 2, 0)).astype(bf16)
    wT = np.ascontiguousarray(W.T).astype(bf16)         # [2H, H]
    bias_r = np.ascontiguousarray(b.reshape(HT, P).T)   # [P, HT]
    v_r = np.ascontiguousarray(v.reshape(HT, P).T)      # [P, HT]

    in_maps = []
    for c in range(NCORES):
        bs = slice(c * BL, (c + 1) * BL)
        h_c = hidden[bs]                                # [BL, H]
        hT_r = np.ascontiguousarray(
            h_c.T.reshape(KT, P, BL).transpose(1, 0, 2)).astype(bf16)
        in_maps.append({
            "eoT": eoT[bs],
            "wT": wT,
            "hTr": hT_r,
            "biasr": bias_r,
            "vr": v_r,
            "mask": mask_u8[bs],
        })
    return in_maps


def run(hidden, encoder_outputs, encoder_mask, W, b, v, trace=False):
    from concourse.bass_utils import run_bass_kernel_spmd

    nc = _get_nc()
    in_maps = _make_in_maps(hidden, encoder_outputs, encoder_mask, W, b, v)
    res = run_bass_kernel_spmd(nc, in_maps, core_ids=list(range(NCORES)),
                               trace=trace)
    out = np.concatenate([res.results[c]["out"] for c in range(NCORES)],
                         axis=0)
    return out.reshape(B, 1, S).astype(np.float32), res


def kernel(hidden, encoder_outputs, encoder_mask, W, b, v):
    out, _ = run(hidden, encoder_outputs, encoder_mask, W, b, v, trace=False)
    return out



# revision 4
# speedup vs baseline: 1.6882x; 1.6882x over previous
"""Bahdanau-attention kernel for 8 TRN2 NeuronCores (fp8 DoubleRow).

Reference computation (B=32, S=2048, H=1024):
    eo   = encoder_outputs.transpose(1,0,2)            # [B,S,H]
    z    = hidden @ W[:, :H].T + eo @ W[:, H:].T + b   # [B,S,H]  (split concat)
    s    = tanh(z)
    sc   = einsum('bsh,h->bs', s, v)
    sc   = where(mask, -1e9, sc); softmax over S       # [B,1,S]

Sharding: data-parallel over batch, 4 batches per core, no collectives.

The dominant matmul (We @ eo, 17.2 GFLOP/core) runs in fp8 e4m3 with
MatmulPerfMode.DoubleRow: lhsT [128, 2, 128] / rhs [128, 2, 256] give an
effective K=256 contraction at 0.5 PE cycles per output column -- 2x the
bf16 rate.  We is scaled x8192 and eo x16 on the host (both fit e4m3's
+-240 range); the tanh activation applies scale=2^-17 to undo it.

The hidden-path pre-activation pre[b,h] = hidden @ Wh^T + bias is
computed on the host in float64 and shipped as the tanh's per-partition
bias (16 KB/core) -- removes both the on-device Wh matmul and its bf16
quantization error (the fp8 main path needs the error margin).

Per (b, s-tile of 1024): 8 h-groups of 16 DoubleRow matmuls into a
2-bank psum tile, tanh+bias on ScalarE -> bf16, v-weighted accumulate
on VectorE (4x/2x DVE modes), partition-reduce via ones-matmul with the
-60000 mask row folded in as a K=1 bf16 matmul, exp on ScalarE with
accumulated row-sum, rolling normalize under later tiles' compute.
"""

import sys

if "/opt/trn_rl_repo" not in sys.path:
    sys.path.insert(0, "/opt/trn_rl_repo")

import numpy as np

B, S, H = 32, 2048, 1024
NCORES = 8
BL = B // NCORES          # batches per core = 4
P = 128                   # partitions
KT = H // P               # k-tiles over the contraction dim = 8
HT = H // P               # h-tiles over the attn output dim = 8
ST = 1024                 # s-tile width (2 psum banks in fp32)
NS = S // ST              # s-tiles per batch = 2
NSJ = ST // 256           # 256-col matmul chunks per s-tile = 4
SW = 8192.0               # host-side We scale before e4m3 quantization
SE = 16.0                 # host-side eo scale before e4m3 quantization
DESCALE = 1.0 / (SW * SE)
MASK_NEG = -60000.0       # additive mask: exp(sc - 60000) == 0

_compiled_nc = None


def _build():
    import concourse.mybir as mybir
    from concourse import tile, bacc
    from concourse.tile import add_dep_helper

    f32 = mybir.dt.float32
    bf16 = mybir.dt.bfloat16
    fp8 = mybir.dt.float8e4
    AF = mybir.ActivationFunctionType
    ALU = mybir.AluOpType
    AX = mybir.AxisListType
    DR = mybir.MatmulPerfMode.DoubleRow

    nc = bacc.Bacc("TRN2", target_bir_lowering=False, debug=False,
                   num_devices=NCORES)

    eoT = nc.dram_tensor("eoT", [BL, H, S], fp8, kind="ExternalInput")
    weT = nc.dram_tensor("weT", [P, KT, H], fp8, kind="ExternalInput")
    prer = nc.dram_tensor("prer", [P, HT * BL], f32, kind="ExternalInput")
    vr = nc.dram_tensor("vr", [P, HT], f32, kind="ExternalInput")
    mneg = nc.dram_tensor("mneg", [1, BL * S], bf16, kind="ExternalInput")
    out = nc.dram_tensor("out", [BL, S], f32, kind="ExternalOutput")

    with tile.TileContext(nc) as tc:
        with (
            tc.tile_pool(name="const", bufs=1) as const,
            tc.tile_pool(name="eo", bufs=4) as eo_pool,
            tc.tile_pool(name="tpool", bufs=4) as t_pool,
            tc.tile_pool(name="tvpool", bufs=3) as tv_pool,
            tc.tile_pool(name="accpool", bufs=3) as acc_pool,
            tc.tile_pool(name="erow", bufs=3) as e_pool,
            tc.tile_pool(name="psz", bufs=3, space="PSUM") as psum_z,
            tc.tile_pool(name="pss", bufs=1, space="PSUM") as psum_s,
        ):
            # --- tiny consts first on the HWDGE ring ---
            pre_sb = const.tile([P, HT * BL], f32)
            nc.sync.dma_start(pre_sb[:], prer[:, :])
            v_sb = const.tile([P, HT], f32)
            nc.sync.dma_start(v_sb[:], vr[:, :])
            mneg_sb = const.tile([1, BL * S], bf16)
            nc.sync.dma_start(mneg_sb[:], mneg[:, :])

            ones_sb = const.tile([P, 1], bf16)
            nc.any.memset(ones_sb[:], 1.0)
            junk = const.tile([P, 512], bf16)
            nc.vector.memset(junk[:], 1.0)

            # weights ride HWDGE (two queues), first eo tile rides SWDGE
            # concurrently; later eo prefetches chain behind the weight DMA
            # so the head window splits HBM bandwidth between few streams.
            eo_first = eo_pool.tile([P, KT, ST], fp8, tag="eo")
            nc.gpsimd.dma_start(
                eo_first[:],
                eoT[0, :, 0:ST].rearrange("(kk p) s -> p kk s", p=P))
            we_sb = const.tile([P, KT, H], fp8)
            d_we0 = nc.sync.dma_start(
                we_sb[:, 0:KT // 2, :], weT[:, 0:KT // 2, :])
            nc.sync.dma_start(
                we_sb[:, KT // 2:KT, :], weT[:, KT // 2:KT, :])
            _dma_chain = [d_we0]

            # PE warmup: dummy matmuls ride out the HAM cold window and the
            # p-state ramp while weights/eo stream in.
            wps = psum_z.tile([P, ST], f32, tag="psz")
            for w in range(28):
                nc.tensor.matmul(wps[:, 0:512], junk[:, 0:P], junk[:],
                                 start=(w == 0), stop=(w == 27),
                                 skip_group_check=True)

            e_sb = const.tile([BL, S], f32)
            o_sb = const.tile([BL, S], f32)
            red_row = const.tile([1, BL * NS], f32)
            psums2 = const.tile([BL, NS], f32)

            for b in range(BL):
                for st in range(NS):
                    if b == 0 and st == 0:
                        eo_sb = eo_first
                    else:
                        eo_sb = eo_pool.tile([P, KT, ST], fp8, tag="eo")
                        d_eo = nc.gpsimd.dma_start(
                            eo_sb[:],
                            eoT[b, :, st * ST:(st + 1) * ST].rearrange(
                                "(kk p) s -> p kk s", p=P))
                        if len(_dma_chain) < 3:
                            add_dep_helper(d_eo.ins, _dma_chain[-1].ins, True,
                                           "serial head dma")
                            _dma_chain.append(d_eo)
                    acc = acc_pool.tile([P, ST], bf16, tag="acc")
                    for hh in range(HT):
                        ps = psum_z.tile([P, ST], f32, tag="psz")
                        for kj in range(KT // 2):
                            wsl = we_sb[:, 2 * kj:2 * kj + 2,
                                        hh * P:(hh + 1) * P]
                            for sj in range(NSJ):
                                # psum "start" zeroes a whole 2KB bank (2
                                # 256-col quarters): only the first matmul
                                # of each bank starts; the second quarter's
                                # first write lands on pending-zero bytes.
                                nc.tensor.matmul(
                                    ps[:, sj * 256:(sj + 1) * 256],
                                    wsl,
                                    eo_sb[:, 2 * kj:2 * kj + 2,
                                          sj * 256:(sj + 1) * 256],
                                    start=(kj == 0 and sj % 2 == 0),
                                    stop=(kj == KT // 2 - 1 and sj % 2 == 1),
                                    perf_mode=DR, skip_group_check=True)
                        t_sb = t_pool.tile([P, ST], bf16, tag="t")
                        nc.scalar.activation(
                            t_sb[:], ps[:], AF.Tanh,
                            bias=pre_sb[:, hh * BL + b:hh * BL + b + 1],
                            scale=DESCALE)
                        if hh == 0:
                            nc.vector.tensor_scalar(acc[:], t_sb[:],
                                                    v_sb[:, 0:1], None,
                                                    ALU.mult)
                        else:
                            tv = tv_pool.tile([P, ST], bf16, tag="tv")
                            nc.vector.tensor_scalar(tv[:], t_sb[:],
                                                    v_sb[:, hh:hh + 1],
                                                    None, ALU.mult)
                            nc.vector.tensor_tensor(acc[:], acc[:], tv[:],
                                                    ALU.add)
                    # --- score flush for (b, st) ---
                    pssc = psum_s.tile([P, ST], f32, tag="pss")
                    for half in range(2):
                        sl = slice(half * 512, (half + 1) * 512)
                        off = b * S + st * ST + half * 512
                        nc.tensor.matmul(pssc[:1, sl], ones_sb[:],
                                         acc[:, sl], start=True, stop=False,
                                         skip_group_check=True)
                        nc.tensor.matmul(pssc[:1, sl], ones_sb[0:1, :],
                                         mneg_sb[:1, off:off + 512],
                                         start=False, stop=True,
                                         skip_group_check=True)
                    e_row = e_pool.tile([1, ST], f32, tag="e")
                    idx = b * NS + st
                    nc.scalar.activation(e_row[:], pssc[:1, :], AF.Exp,
                                         accum_out=red_row[:, idx:idx + 1])
                    nc.sync.dma_start(
                        e_sb[b:b + 1, st * ST:(st + 1) * ST], e_row[:])
                    nc.sync.dma_start(psums2[b:b + 1, st:st + 1],
                                      red_row[:, idx:idx + 1])
                    if b == BL - 2 and st == NS - 1:
                        # batches 0-2 complete: normalize + store their rows
                        # under batch 3's compute
                        r3 = const.tile([BL - 1, 1], f32)
                        nc.vector.reduce_sum(r3[:], psums2[0:BL - 1, :],
                                             axis=AX.X)
                        nc.vector.reciprocal(r3[:], r3[:])
                        nc.vector.tensor_scalar(o_sb[0:BL - 1, :],
                                                e_sb[0:BL - 1, :], r3[:],
                                                None, ALU.mult)
                        nc.sync.dma_start(out[0:BL - 1, :], o_sb[0:BL - 1, :])

            # tail: only batch 3 left. Engine ops start at partition 0, so
            # compute [BL, S] (rows 0-2 recompute identically) but store
            # only row 3, chunked to overlap the DMA.
            rinv = const.tile([BL, 1], f32)
            nc.vector.reduce_sum(rinv[:], psums2[:], axis=AX.X)
            nc.vector.reciprocal(rinv[:], rinv[:])
            for ci in range(4):
                cs = slice(ci * (S // 4), (ci + 1) * (S // 4))
                nc.vector.tensor_scalar(o_sb[:, cs], e_sb[:, cs], rinv[:],
                                        None, ALU.mult)
                nc.sync.dma_start(out[BL - 1:BL, cs], o_sb[BL - 1:BL, cs])

    nc.compile()
    return nc


def _get_nc():
    global _compiled_nc
    if _compiled_nc is None:
        _compiled_nc = _build()
    return _compiled_nc


def _make_in_maps(hidden, encoder_outputs, encoder_mask, W, b, v):
    import ml_dtypes

    bf16 = ml_dtypes.bfloat16
    e4m3 = ml_dtypes.float8_e4m3   # mybir float8e4 <-> IEEE e4m3 (max 240)
    hidden = np.asarray(hidden, dtype=np.float32)
    encoder_outputs = np.asarray(encoder_outputs, dtype=np.float32)
    W = np.asarray(W, dtype=np.float32)
    b = np.asarray(b, dtype=np.float32)
    v = np.asarray(v, dtype=np.float32)
    mask_u8 = np.asarray(encoder_mask).reshape(B, S).astype(np.uint8)

    # [S, B, H] -> [B, H, S], scaled x16 into e4m3 (|eo| < 6 sigma -> < 96)
    eoT = np.ascontiguousarray(
        encoder_outputs.transpose(1, 2, 0) * SE).astype(e4m3)
    # We^T x8192 (|We| <= 1/sqrt(2H) -> max 181 < 240), [P, KT, H]
    weT = np.ascontiguousarray(
        (W[:, H:].T * SW).reshape(KT, P, H).transpose(1, 0, 2)).astype(e4m3)
    # hidden-path pre-activation in float64 on host
    pre = (hidden.astype(np.float64) @ W[:, :H].astype(np.float64).T
           + b.astype(np.float64)).astype(np.float32)        # [B, H]
    v_r = np.ascontiguousarray(v.reshape(HT, P).T)           # [P, HT]
    mneg_f = mask_u8.astype(np.float32) * np.float32(MASK_NEG)

    in_maps = []
    for c in range(NCORES):
        bs = slice(c * BL, (c + 1) * BL)
        pre_c = np.ascontiguousarray(
            pre[bs].T.reshape(HT, P, BL).transpose(1, 0, 2).reshape(
                P, HT * BL))
        in_maps.append({
            "eoT": eoT[bs],
            "weT": weT,
            "prer": pre_c,
            "vr": v_r,
            "mneg": mneg_f[bs].reshape(1, BL * S).astype(bf16),
        })
    return in_maps


def run(hidden, encoder_outputs, encoder_mask, W, b, v, trace=False):
    from concourse.bass_utils import run_bass_kernel_spmd

    nc = _get_nc()
    in_maps = _make_in_maps(hidden, encoder_outputs, encoder_mask, W, b, v)
    res = run_bass_kernel_spmd(nc, in_maps, core_ids=list(range(NCORES)),
                               trace=trace)
    out = np.concatenate([res.results[c]["out"] for c in range(NCORES)],
                         axis=0)
    return out.reshape(B, 1, S).astype(np.float32), res


def kernel(hidden, encoder_outputs, encoder_mask, W, b, v):
    out, _ = run(hidden, encoder_outputs, encoder_mask, W, b, v, trace=False)
    return out


# revision 9
# speedup vs baseline: 1.7033x; 1.0090x over previous
"""Bahdanau-attention kernel for 8 TRN2 NeuronCores (fp8 DoubleRow).

Reference computation (B=32, S=2048, H=1024):
    eo   = encoder_outputs.transpose(1,0,2)            # [B,S,H]
    z    = hidden @ W[:, :H].T + eo @ W[:, H:].T + b   # [B,S,H]  (split concat)
    s    = tanh(z)
    sc   = einsum('bsh,h->bs', s, v)
    sc   = where(mask, -1e9, sc); softmax over S       # [B,1,S]

Sharding: data-parallel over batch, 4 batches per core, no collectives.

The dominant matmul (We @ eo, 17.2 GFLOP/core) runs in fp8 e4m3 with
MatmulPerfMode.DoubleRow: lhsT [128, 2, 128] / rhs [128, 2, 256] give an
effective K=256 contraction at 0.5 PE cycles per output column -- 2x the
bf16 rate.  We is scaled x8192 and eo x16 on the host (both fit e4m3's
+-240 range); the tanh activation applies scale=2^-17 to undo it.

The hidden-path pre-activation pre[b,h] = hidden @ Wh^T + bias is
computed on the host in float64 and shipped as the tanh's per-partition
bias (16 KB/core) -- removes both the on-device Wh matmul and its bf16
quantization error (the fp8 main path needs the error margin).

Per (b, s-tile of 1024): 8 h-groups of 16 DoubleRow matmuls into a
2-bank psum tile, tanh+bias on ScalarE -> bf16, v-weighted accumulate
on VectorE (4x/2x DVE modes), partition-reduce via ones-matmul with the
-60000 mask row folded in as a K=1 bf16 matmul, exp on ScalarE with
accumulated row-sum, rolling normalize under later tiles' compute.
"""

import sys

if "/opt/trn_rl_repo" not in sys.path:
    sys.path.insert(0, "/opt/trn_rl_repo")

import numpy as np

B, S, H = 32, 2048, 1024
NCORES = 8
BL = B // NCORES          # batches per core = 4
P = 128                   # partitions
KT = H // P               # k-tiles over the contraction dim = 8
HT = H // P               # h-tiles over the attn output dim = 8
ST = 1024                 # s-tile width (2 psum banks in fp32)
NS = S // ST              # s-tiles per batch = 2
NSJ = ST // 256           # 256-col matmul chunks per s-tile = 4
SW = 8192.0               # host-side We scale before e4m3 quantization
SE = 16.0                 # host-side eo scale before e4m3 quantization
DESCALE = 1.0 / (SW * SE)
MASK_NEG = -60000.0       # additive mask: exp(sc - 60000) == 0

_compiled_nc = None


def _build():
    import concourse.mybir as mybir
    from concourse import tile, bacc
    from concourse.tile import add_dep_helper

    f32 = mybir.dt.float32
    bf16 = mybir.dt.bfloat16
    fp16 = mybir.dt.float16
    fp8 = mybir.dt.float8e4
    AF = mybir.ActivationFunctionType
    ALU = mybir.AluOpType
    AX = mybir.AxisListType
    DR = mybir.MatmulPerfMode.DoubleRow

    nc = bacc.Bacc("TRN2", target_bir_lowering=False, debug=False,
                   num_devices=NCORES)

    eoT = nc.dram_tensor("eoT", [BL, H, S], fp8, kind="ExternalInput")
    weT = nc.dram_tensor("weT", [P, KT, H], fp8, kind="ExternalInput")
    prer = nc.dram_tensor("prer", [P, HT * BL], f32, kind="ExternalInput")
    vr = nc.dram_tensor("vr", [P, HT], f32, kind="ExternalInput")
    mneg = nc.dram_tensor("mneg", [1, BL * S], fp16, kind="ExternalInput")
    out = nc.dram_tensor("out", [BL, S], f32, kind="ExternalOutput")

    with tile.TileContext(nc) as tc:
        with (
            tc.tile_pool(name="const", bufs=1) as const,
            tc.tile_pool(name="eo", bufs=4) as eo_pool,
            tc.tile_pool(name="tpool", bufs=4) as t_pool,
            tc.tile_pool(name="tvpool", bufs=3) as tv_pool,
            tc.tile_pool(name="accpool", bufs=3) as acc_pool,
            tc.tile_pool(name="erow", bufs=3) as e_pool,
            tc.tile_pool(name="psz", bufs=3, space="PSUM") as psum_z,
            tc.tile_pool(name="pss", bufs=1, space="PSUM") as psum_s,
        ):
            # --- tiny consts first on the HWDGE ring ---
            pre_sb = const.tile([P, HT * BL], f32)
            nc.sync.dma_start(pre_sb[:], prer[:, :])
            v_sb = const.tile([P, HT], f32)
            nc.sync.dma_start(v_sb[:], vr[:, :])
            mneg_sb = const.tile([1, BL * S], fp16)
            nc.sync.dma_start(mneg_sb[:], mneg[:, :])

            ones_sb = const.tile([P, 1], fp16)
            nc.any.memset(ones_sb[:], 1.0)
            junk = const.tile([P, 512], bf16)
            nc.vector.memset(junk[:], 1.0)

            # weights ride HWDGE (two queues), first eo tile rides SWDGE
            # concurrently; later eo prefetches chain behind the weight DMA
            # so the head window splits HBM bandwidth between few streams.
            eo_first = eo_pool.tile([P, KT, ST], fp8, tag="eo")
            nc.gpsimd.dma_start(
                eo_first[:],
                eoT[0, :, 0:ST].rearrange("(kk p) s -> p kk s", p=P))
            we_sb = const.tile([P, KT, H], fp8)
            d_we0 = nc.sync.dma_start(
                we_sb[:, 0:KT // 2, :], weT[:, 0:KT // 2, :])
            nc.sync.dma_start(
                we_sb[:, KT // 2:KT, :], weT[:, KT // 2:KT, :])
            _dma_chain = [d_we0]

            # PE warmup: dummy matmuls ride out the HAM cold window and the
            # p-state ramp while weights/eo stream in.
            wps = psum_z.tile([P, ST], f32, tag="psz")
            for w in range(14):
                nc.tensor.matmul(wps[:, 0:512], junk[:, 0:P], junk[:],
                                 start=(w == 0), stop=(w == 13),
                                 skip_group_check=True)

            e_sb = const.tile([BL, S], f32)
            o_sb = const.tile([BL, S], f32)
            red_row = const.tile([1, BL * NS], f32)
            psums2 = const.tile([BL, NS], f32)

            n_flushed = [0]

            def flush_scores(acc_p, b_p, st_p):
                # partition-reduce acc via ones-matmul (mask already folded
                # into acc row 0), exp + row-sum, rolling normalize.
                pssc = psum_s.tile([P, ST], f32, tag="pss")
                for half in range(2):
                    sl = slice(half * 512, (half + 1) * 512)
                    nc.tensor.matmul(pssc[:1, sl], ones_sb[:],
                                     acc_p[:, sl], start=True, stop=True,
                                     skip_group_check=True)
                e_row = e_pool.tile([1, ST], f32, tag="e")
                idx = b_p * NS + st_p
                nc.scalar.activation(e_row[:], pssc[:1, :], AF.Exp,
                                     accum_out=red_row[:, idx:idx + 1])
                nc.sync.dma_start(
                    e_sb[b_p:b_p + 1, st_p * ST:(st_p + 1) * ST], e_row[:])
                nc.sync.dma_start(psums2[b_p:b_p + 1, st_p:st_p + 1],
                                  red_row[:, idx:idx + 1])
                n_flushed[0] += 1
                if n_flushed[0] == (BL - 1) * NS:
                    # batches 0-2 complete: normalize + store their rows
                    # under batch 3's compute
                    r3 = const.tile([BL - 1, 1], f32)
                    nc.vector.reduce_sum(r3[:], psums2[0:BL - 1, :],
                                         axis=AX.X)
                    nc.vector.reciprocal(r3[:], r3[:])
                    nc.vector.tensor_scalar(o_sb[0:BL - 1, :],
                                            e_sb[0:BL - 1, :], r3[:],
                                            None, ALU.mult)
                    nc.sync.dma_start(out[0:BL - 1, :], o_sb[0:BL - 1, :])

            pending = []
            for b in range(BL):
                for st in range(NS):
                    if b == 0 and st == 0:
                        eo_sb = eo_first
                    else:
                        eo_sb = eo_pool.tile([P, KT, ST], fp8, tag="eo")
                        d_eo = nc.gpsimd.dma_start(
                            eo_sb[:],
                            eoT[b, :, st * ST:(st + 1) * ST].rearrange(
                                "(kk p) s -> p kk s", p=P))
                        if len(_dma_chain) < 3:
                            add_dep_helper(d_eo.ins, _dma_chain[-1].ins, True,
                                           "serial head dma")
                            _dma_chain.append(d_eo)
                    acc = acc_pool.tile([P, ST], fp16, tag="acc")
                    for hh in range(HT):
                        ps = psum_z.tile([P, ST], f32, tag="psz")
                        for kj in range(KT // 2):
                            wsl = we_sb[:, 2 * kj:2 * kj + 2,
                                        hh * P:(hh + 1) * P]
                            for sj in range(NSJ):
                                # psum "start" zeroes a whole 2KB bank (2
                                # 256-col quarters): only the first matmul
                                # of each bank starts; the second quarter's
                                # first write lands on pending-zero bytes.
                                nc.tensor.matmul(
                                    ps[:, sj * 256:(sj + 1) * 256],
                                    wsl,
                                    eo_sb[:, 2 * kj:2 * kj + 2,
                                          sj * 256:(sj + 1) * 256],
                                    start=(kj == 0 and sj % 2 == 0),
                                    stop=(kj == KT // 2 - 1 and sj % 2 == 1),
                                    perf_mode=DR, skip_group_check=True)
                        if hh == 2 and pending:
                            # flush the previous tile's scores here: its DVE
                            # accumulate chain finished during hh 0-1, so the
                            # PE never stalls waiting on it.
                            flush_scores(*pending.pop())
                        t_sb = t_pool.tile([P, ST], fp16, tag="t")
                        nc.scalar.activation(
                            t_sb[:], ps[:], AF.Tanh,
                            bias=pre_sb[:, hh * BL + b:hh * BL + b + 1],
                            scale=DESCALE)
                        if hh == 0:
                            nc.vector.tensor_scalar(acc[:], t_sb[:],
                                                    v_sb[:, 0:1], None,
                                                    ALU.mult)
                        else:
                            tv = tv_pool.tile([P, ST], fp16, tag="tv")
                            nc.vector.tensor_scalar(tv[:], t_sb[:],
                                                    v_sb[:, hh:hh + 1],
                                                    None, ALU.mult)
                            nc.vector.tensor_tensor(acc[:], acc[:], tv[:],
                                                    ALU.add)
                    # fold the -60000 mask row into acc partition 0 (fp16,
                    # DVE): the ones-reduction then includes it for free.
                    off = b * S + st * ST
                    nc.vector.tensor_tensor(acc[0:1, :], acc[0:1, :],
                                            mneg_sb[:1, off:off + ST],
                                            ALU.add)
                    pending.append((acc, b, st))
            flush_scores(*pending.pop())

            # tail: only batch 3 left. Engine ops start at partition 0, so
            # compute [BL, S] (rows 0-2 recompute identically) but store
            # only row 3, chunked to overlap the DMA.
            rinv = const.tile([BL, 1], f32)
            nc.vector.reduce_sum(rinv[:], psums2[:], axis=AX.X)
            nc.vector.reciprocal(rinv[:], rinv[:])
            for ci in range(4):
                cs = slice(ci * (S // 4), (ci + 1) * (S // 4))
                nc.vector.tensor_scalar(o_sb[:, cs], e_sb[:, cs], rinv[:],
                                        None, ALU.mult)
                nc.sync.dma_start(out[BL - 1:BL, cs], o_sb[BL - 1:BL, cs])

    nc.compile()
    return nc


def _get_nc():
    global _compiled_nc
    if _compiled_nc is None:
        _compiled_nc = _build()
    return _compiled_nc


def _make_in_maps(hidden, encoder_outputs, encoder_mask, W, b, v):
    import ml_dtypes

    bf16 = ml_dtypes.bfloat16
    e4m3 = ml_dtypes.float8_e4m3   # mybir float8e4 <-> IEEE e4m3 (max 240)
    hidden = np.asarray(hidden, dtype=np.float32)
    encoder_outputs = np.asarray(encoder_outputs, dtype=np.float32)
    W = np.asarray(W, dtype=np.float32)
    b = np.asarray(b, dtype=np.float32)
    v = np.asarray(v, dtype=np.float32)
    mask_u8 = np.asarray(encoder_mask).reshape(B, S).astype(np.uint8)

    # [S, B, H] -> [B, H, S], scaled x16 into e4m3 (|eo| < 6 sigma -> < 96)
    eoT = np.ascontiguousarray(
        encoder_outputs.transpose(1, 2, 0) * SE).astype(e4m3)
    # We^T x8192 (|We| <= 1/sqrt(2H) -> max 181 < 240), [P, KT, H]
    weT = np.ascontiguousarray(
        (W[:, H:].T * SW).reshape(KT, P, H).transpose(1, 0, 2)).astype(e4m3)
    # hidden-path pre-activation in float64 on host
    pre = (hidden.astype(np.float64) @ W[:, :H].astype(np.float64).T
           + b.astype(np.float64)).astype(np.float32)        # [B, H]
    v_r = np.ascontiguousarray(v.reshape(HT, P).T)           # [P, HT]
    mneg_f = mask_u8.astype(np.float32) * np.float32(MASK_NEG)

    in_maps = []
    for c in range(NCORES):
        bs = slice(c * BL, (c + 1) * BL)
        pre_c = np.ascontiguousarray(
            pre[bs].T.reshape(HT, P, BL).transpose(1, 0, 2).reshape(
                P, HT * BL))
        in_maps.append({
            "eoT": eoT[bs],
            "weT": weT,
            "prer": pre_c,
            "vr": v_r,
            "mneg": mneg_f[bs].reshape(1, BL * S).astype(np.float16),
        })
    return in_maps


def run(hidden, encoder_outputs, encoder_mask, W, b, v, trace=False):
    from concourse.bass_utils import run_bass_kernel_spmd

    nc = _get_nc()
    in_maps = _make_in_maps(hidden, encoder_outputs, encoder_mask, W, b, v)
    res = run_bass_kernel_spmd(nc, in_maps, core_ids=list(range(NCORES)),
                               trace=trace)
    out = np.concatenate([res.results[c]["out"] for c in range(NCORES)],
                         axis=0)
    return out.reshape(B, 1, S).astype(np.float32), res


def kernel(hidden, encoder_outputs, encoder_mask, W, b, v):
    out, _ = run(hidden, encoder_outputs, encoder_mask, W, b, v, trace=False)
    return out
